# revision 1
# baseline (speedup 1.0000x reference)
"""Trainium2 Bass kernel: end-to-end model (pool -> linear -> max/argmax ->
top-k -> gather), data-parallel over 8 NeuronCores (batch sharded).

Self-contained: hardcodes all shapes; builds one SPMD Bass program and runs
it via run_bass_kernel_spmd on cores 0-7.
"""

import os
import sys
from contextlib import ExitStack

import numpy as np

for _p in ("/opt/trn_rl_repo", "/root/.axon_site/_ro/trn_rl_repo"):
    if os.path.isdir(_p) and _p not in sys.path:
        sys.path.append(_p)

import concourse.bass as bass
import concourse.tile as tile
from concourse import bacc, library_config, mybir
from concourse.bass_utils import run_bass_kernel_spmd

dt = mybir.dt
F32 = dt.float32
AX = mybir.AxisListType
OP = mybir.AluOpType

# ---------------- problem constants (hardcoded) ----------------
B, CHN, HIMG, WIMG = 64, 3, 640, 640
NQ, NCHAN, NCL, TOPK = 300, 84, 80, 150
KDIM, NOUT = 1200, 25200           # 3*20*20, NQ*NCHAN
NCORES = 8
BPC = B // NCORES                  # samples per core = 8
KT, KTS = 10, 120                  # main matmul K tiling: 10 x 120
QPC = 6                            # queries per N chunk
NCHUNK = QPC * NCHAN               # 504 columns per chunk (<=512 psum bank)
NCHUNKS = NOUT // NCHUNK           # 50
SCALE = np.float64(1.0) / (32 * 32 * 255)
NEG = -3.0e38
NIDX = 160                         # padded top-k index count (152 used)
NROUND = 19                        # 19 rounds x 8 = 152 >= 150

# matmul input dtype: float32 (exact-ish) or float32r (fast, reduced precision)
MM_DTYPE = dt.float32r if os.environ.get("KERNEL_F32R", "0") == "1" else F32


GRP = 2  # chunks per W fetch group (divides 50)
NGRP = NCHUNKS // GRP


def build_program():
    # Bacc (not raw Bass): its compile() splits multi-sem waits (TRN2 allows
    # one wait per instruction), auto-inserts gpsimd library loads, and
    # lowers extended-ISA instructions (ap_gather) to bytes.
    # x and w arrive HOST-PACKED into tile-contiguous layouts so every DMA
    # is a linear read (strided 4KB rows only reach ~250GB/s of HBM).
    nc = bacc.Bacc("TRN2", target_bir_lowering=False, debug=False)
    # x packed to uint8 on the host (values are 0..255): 4x fewer HBM bytes
    x_d = nc.dram_tensor(
        "x", [BPC, CHN, 128, 5, WIMG], dt.uint8, kind="ExternalInput"
    )
    w_d = nc.dram_tensor(
        "w", [KT, NGRP, KTS, GRP * NCHUNK], F32, kind="ExternalInput"
    )
    g4_d = nc.dram_tensor("g4", [128, 4], F32, kind="ExternalInput")
    id8_d = nc.dram_tensor("id8", [BPC, BPC], F32, kind="ExternalInput")
    iod_d = nc.dram_tensor("iod", [128, NCL], F32, kind="ExternalInput")
    out_d = nc.dram_tensor("out", [BPC, TOPK, 6], F32, kind="ExternalOutput")

    with tile.TileContext(nc) as tc:
        with ExitStack() as ctx:
            _body(ctx, tc, x_d, w_d, g4_d, id8_d, iod_d, out_d)
    nc.finalize()
    return nc


def _body(ctx, tc, x_d, w_d, g4_d, id8_d, iod_d, out_d):
    nc = tc.nc

    # ---------------- persistent tiles ----------------
    P = ctx.enter_context(tc.tile_pool(name="persist", bufs=1))

    g4 = P.tile([128, 4], F32, tag="g4")
    nc.sync.dma_start(g4[:], g4_d[:])
    id8 = P.tile([BPC, BPC], F32, tag="id8")
    nc.sync.dma_start(id8[:], id8_d[:])
    iod = P.tile([128, NCL], F32, tag="iod")
    nc.sync.dma_start(iod[:], iod_d[:])

    yall = P.tile([BPC, KDIM], F32, tag="yall")          # pooled, per-sample rows
    boxes = P.tile([BPC, NQ * 4], F32, tag="boxes")      # box cols per query
    scores = P.tile([BPC, NQ], F32, tag="scores")        # per-query max score
    swk = P.tile([BPC, NQ], F32, tag="swk")              # topk scratch (destroyed)
    ids = P.tile([BPC, NQ], F32, tag="ids")              # per-query argmax id
    feat = P.tile([128, NQ * 6], F32, tag="feat")        # gather source [p, q, 6]
    tv = P.tile([BPC, NROUND * 8], F32, tag="tv")        # topk values (desc)
    ti = P.tile([BPC, NROUND * 8], dt.uint32, tag="ti")  # topk indices
    ti16 = P.tile([BPC, NIDX], dt.int16, tag="ti16")
    wrap = P.tile([128, NIDX // 16], dt.int16, tag="wrap")
    gout = P.tile([128, NIDX * 6], F32, tag="gout")
    pt = [P.tile([KTS, BPC], F32, tag=f"pt{k}", name=f"pt{k}") for k in range(KT)]
    ybatch = [P.tile([128, NCHUNK], F32, tag=f"yb{t}", name=f"yb{t}") for t in range(4)]
    # argmax scratch lives in the persistent pool: a separate pool opened
    # after the main loop would reuse the W pool's address range, forcing
    # the whole argmax phase to serialize after the last W access.
    mx = P.tile([128, QPC], F32, tag="mx")
    eq = P.tile([128, QPC * NCL], F32, tag="eq")
    am = P.tile([128, QPC * NCL], F32, tag="am")
    arg = P.tile([128, QPC], F32, tag="arg")
    idt = P.tile([128, QPC], F32, tag="idt")

    nc.vector.memset(ti16[:, :], 0)
    nc.vector.memset(feat[:, :], 0)  # only partitions 16b hold real data
    # load the gather library up front so no drain+reload lands in the tail
    nc.gpsimd.load_library(library_config.ap_gather)

    # ---------------- phase 1: pooling (x -> yall [8,1200]) ----------------
    with tc.tile_pool(name="xp", bufs=4) as XP, \
         tc.tile_pool(name="s1p", bufs=6) as S1P, \
         tc.tile_pool(name="smallp", bufs=6) as SMALL, \
         tc.tile_pool(name="pps", bufs=4, space="PSUM") as PPS, \
         tc.tile_pool(name="pts", bufs=2, space="PSUM") as PTS:
        for cx in (2, 1, 0):  # BGR->RGB handled via destination offset
            for b in range(BPC):  # channel-outer: gate columns finish early
                # two independent tiles per image -> two HW queues in flight
                # and finer-grained reduce pipelining
                xsrc = x_d[b, cx]  # [128, 5, 640], host-packed partition-major
                xa1 = XP.tile([128, 2 * WIMG], dt.uint8, tag="xa1", name="xa1")
                xa2 = XP.tile([128, 3 * WIMG], dt.uint8, tag="xa2", name="xa2")
                nc.sync.dma_start(
                    xa1[:].rearrange("p (t w) -> p t w", t=2), xsrc[:, 0:2, :]
                )
                nc.sync.dma_start(
                    xa2[:].rearrange("p (t w) -> p t w", t=3), xsrc[:, 2:5, :]
                )
                # width pooling: sum groups of 32 -> [128, 5*20] int32
                s1i = S1P.tile([128, 100], dt.int32, tag="s1i", name="s1i")
                with nc.allow_low_precision(reason="int32 sums of uint8 values are exact"):
                    nc.vector.tensor_reduce(
                        s1i[:, 0:40],
                        xa1[:].rearrange("p (t j g) -> p t j g", t=2, j=20),
                        axis=AX.X,
                        op=OP.add,
                    )
                    nc.vector.tensor_reduce(
                        s1i[:, 40:100],
                        xa2[:].rearrange("p (t j g) -> p t j g", t=3, j=20),
                        axis=AX.X,
                        op=OP.add,
                    )
                s1f = S1P.tile([128, 100], F32, tag="s1f", name="s1f")
                nc.vector.tensor_copy(s1f[:], s1i[:])
                # height pooling via PE: G4.T @ s1f -> [4, 100] (scaled)
                ps = PPS.tile([4, 100], F32, tag="ps", name="ps")
                nc.tensor.matmul(ps[:], g4[:], s1f[:], start=True, stop=True)
                pc = SMALL.tile([4, 100], F32, tag="pc", name="pc")
                nc.scalar.copy(pc[:], ps[:])
                # scatter into yall row b at RGB-flipped channel offset
                # k = c*400 + (t*4+i)*20 + j = c*400 + t*80 + i*20 + j
                base = (2 - cx) * 400
                ydst = yall[b : b + 1, base : base + 400].rearrange(
                    "o (t i j) -> o t i j", t=5, i=4
                )
                for i in range(4):
                    # issued on the Scalar queue: keeps the Sync FIFO free of
                    # compute-dependent waits that would stall x DMA issue
                    nc.scalar.dma_start(
                        ydst[:, :, i, :],
                        pc[i : i + 1, :].rearrange("o (t j) -> o t j", t=5),
                    )
        # transpose yall -> pt tiles [120, 8] (lhsT for the main matmul)
        for k in range(KT):
            pst = PTS.tile([KTS, BPC], F32, tag="pst", name="pst")
            nc.tensor.transpose(pst[:], yall[:, k * KTS : (k + 1) * KTS], id8[:])
            nc.vector.tensor_copy(pt[k][:], pst[:])

    # ---------------- phase 2: main matmul + per-chunk postproc ----------------
    # Gate the W stream behind pooling: this dummy read of yall (to a DRAM
    # scratch) stalls the Sync sequencer (and so all later W DMA issues)
    # until pooling is nearly done, giving the x DMAs full HBM bandwidth.
    # Only channels R,G gate (the B tail overlaps the first W fetches).
    with tc.tile_pool(name="dwg", bufs=1, space="DRAM") as DWG:
        # staggered release: half the W stream (sync) starts after 1/3 of
        # pooling, the other half (scalar) after 2/3 — smooths the x->W
        # bandwidth handoff so the PE ramp is fed sooner
        wgate = DWG.tile([BPC, 800], F32, tag="wgate")
        nc.sync.dma_start(wgate[:, 0:400], yall[:, 0:400])
        nc.scalar.dma_start(wgate[:, 400:800], yall[:, 400:800])
        del wgate

    # W DMAs fetch GRP n-chunks at once from the host-packed contiguous
    # layout: one linear 968KB read per DMA.
    groups = [(g * GRP, GRP) for g in range(NGRP)]
    with tc.tile_pool(name="wp", bufs=22) as WP, \
         tc.tile_pool(name="ycp", bufs=4) as YCP, \
         tc.tile_pool(name="yps", bufs=8, space="PSUM") as YPS:
        for gi, (n0, gn) in enumerate(groups):
            wts = []
            for k in range(KT):
                wt = WP.tile([KTS, GRP * NCHUNK], F32, tag="wt", name="wt")
                # alternate between the two HWDGE engines for 2x issue rate
                eng = nc.sync if (gi * KT + k) % 2 == 0 else nc.scalar
                eng.dma_start(wt[:], w_d[k, gi])
                wts.append(wt)
            for j in range(gn):
                n = n0 + j
                psy = YPS.tile([BPC, NCHUNK], F32, tag="psy", name="psy")
                for k in range(KT):
                    nc.tensor.matmul(
                        psy[:],
                        pt[k][:].bitcast(MM_DTYPE),
                        wts[k][:, j * NCHUNK : (j + 1) * NCHUNK].bitcast(MM_DTYPE),
                        start=(k == 0),
                        stop=(k == KT - 1),
                    )
                psv = psy[:].rearrange("b (q c) -> b q c", q=QPC)
                # box columns -> boxes[8, 300*4]  (DVE: Scalar queue is busy
                # issuing W DMAs, and psum release must be prompt)
                nc.vector.tensor_copy(
                    boxes[:, n * 24 : (n + 1) * 24].rearrange("b (q c) -> b q c", c=4),
                    psv[:, :, 0:4],
                )
                # per-query max score -> scores[8, 300]
                nc.vector.tensor_reduce(
                    scores[:, n * QPC : (n + 1) * QPC], psv[:, :, 4:NCHAN],
                    axis=AX.X, op=OP.max,
                )
                # stage full chunk for batched argmax: SBUF copy + partition-move
                yc = YCP.tile([BPC, NCHUNK], F32, tag="yc", name="yc")
                nc.vector.tensor_copy(yc[:], psy[:])
                t, s = divmod(n, 16)
                # NOT gpsimd: SWDGE insts on the Pool engine force a library
                # reload (and a ~10us drain) before the tail's ap_gather
                nc.scalar.dma_start(ybatch[t][8 * s : 8 * s + 8, :], yc[:])

    # ---------------- phase 3: batched argmax over classes ----------------
    # (runs concurrently with the tail of the main loop: batch t is ready
    # once its 16 chunks have been staged)
    for t in range(4):
        pcnt = 128 if t < 3 else 16
        ybv = ybatch[t][:pcnt, :].rearrange("p (q c) -> p q c", q=QPC)[:, :, 4:NCHAN]
        nc.vector.tensor_reduce(mx[:pcnt, :], ybv, axis=AX.X, op=OP.max)
        eqv = eq[:pcnt, :].rearrange("p (q c) -> p q c", q=QPC)
        nc.vector.tensor_tensor(
            eqv, ybv,
            mx[:pcnt, :].unsqueeze(-1).broadcast_to((pcnt, QPC, NCL)),
            op=OP.is_ge,
        )
        amv = am[:pcnt, :].rearrange("p (q c) -> p q c", q=QPC)
        nc.vector.tensor_tensor(
            amv, eqv,
            iod[:pcnt, :].unsqueeze(1).broadcast_to((pcnt, QPC, NCL)),
            op=OP.mult,
        )
        nc.vector.tensor_reduce(arg[:pcnt, :], amv, axis=AX.X, op=OP.max)
        # id = 79 - arg
        nc.vector.tensor_scalar(
            idt[:pcnt, :], arg[:pcnt, :], -1.0, float(NCL - 1),
            op0=OP.mult, op1=OP.add,
        )
        # scatter back to ids[8, 300]
        ns = 16 if t < 3 else 2
        for s in range(ns):
            nc.scalar.dma_start(
                ids[:, (16 * t + s) * QPC : (16 * t + s + 1) * QPC],
                idt[8 * s : 8 * s + 8, :],
            )

    # ---------------- phase 4: feat assembly (gather source) ----------------
    # feat partition 16b holds sample b's rows [q, 6] = [box0..3, score, id];
    # d=6 gather keeps output rows contiguous for a single linear out-DMA.
    for b in range(BPC):
        fview = feat[16 * b : 16 * b + 1, :].rearrange("o (q c) -> o q c", c=6)
        nc.scalar.dma_start(
            fview[:, :, 0:4],
            boxes[b : b + 1, :].rearrange("o (q c) -> o q c", c=4),
        )
        nc.scalar.dma_start(fview[:, :, 4], scores[b : b + 1, :])
        nc.scalar.dma_start(fview[:, :, 5], ids[b : b + 1, :])

    # ---------------- phase 5: top-150 via iterated max8 ----------------
    # work on a copy so the destructive match_replace doesn't serialize
    # against the feat DMAs reading `scores`
    nc.vector.tensor_copy(swk[:, :], scores[:, :])
    for r in range(NROUND):
        nc.vector.max(tv[:, 8 * r : 8 * r + 8], swk[:, :])
        nc.vector.max_index(ti[:, 8 * r : 8 * r + 8], tv[:, 8 * r : 8 * r + 8], swk[:, :])
        if r < NROUND - 1:
            nc.vector.match_replace(
                swk[:, :], tv[:, 8 * r : 8 * r + 8], swk[:, :], NEG
            )

    nc.vector.tensor_copy(ti16[:, : NROUND * 8], ti[:, :])

    # wrap indices into per-core [16, 10] layout (via DRAM — SBUF->SBUF
    # one-to-many partition scatters exceed the 3-dim DMA limit)
    with tc.tile_pool(name="dscr", bufs=1, space="DRAM") as DSCR:
        # DRAM scratch laid out exactly as wrap's rows: (b, p, f)
        tsc = DSCR.tile([BPC, NIDX], dt.int16, tag="tsc")
        nc.scalar.dma_start(tsc[:], ti16[:])
        for b in range(BPC):
            eng = nc.sync if b % 2 == 0 else nc.scalar
            eng.dma_start(
                wrap[16 * b : 16 * b + 16, :],
                tsc[b].rearrange("(f p) -> p f", p=16),
            )

    # ---------------- phase 6: gather + output ----------------
    nc.gpsimd.ap_gather(
        gout[:].rearrange("p (i c) -> p i c", c=6),
        feat[:].rearrange("p (q c) -> p q c", c=6),
        wrap[:],
        channels=128,
        num_elems=NQ,
        d=6,
        num_idxs=NIDX,
    )
    for b in range(BPC):
        eng = nc.sync if b % 2 == 0 else nc.scalar
        eng.dma_start(
            out_d[b : b + 1].rearrange("o k c -> o (k c)"),
            gout[16 * b : 16 * b + 1, : TOPK * 6],
        )


def _make_consts():
    g4 = np.zeros((128, 4), np.float32)
    for i in range(4):
        g4[32 * i : 32 * (i + 1), i] = np.float32(SCALE)
    id8 = np.eye(BPC, dtype=np.float32)
    iod = np.broadcast_to(
        (np.float32(NCL - 1) - np.arange(NCL, dtype=np.float32))[None, :], (128, NCL)
    ).copy()
    return g4, id8, iod


_NC_CACHE = {}


def _get_nc():
    key = str(MM_DTYPE)
    if key not in _NC_CACHE:
        _NC_CACHE[key] = build_program()
    return _NC_CACHE[key]


def pack_w(W: np.ndarray) -> np.ndarray:
    """[1200, 25200] -> [KT, NGRP, 120, GRP*504] with each tile contiguous."""
    Wp = W.reshape(KT, KTS, NGRP, GRP * NCHUNK).transpose(0, 2, 1, 3)
    return np.ascontiguousarray(Wp)


def pack_x(xs: np.ndarray) -> np.ndarray:
    """[BPC, 3, 640, 640] int32 -> [BPC, 3, 128, 5, 640] uint8 partition-major."""
    return np.ascontiguousarray(
        xs.reshape(BPC, CHN, 5, 128, WIMG).transpose(0, 1, 3, 2, 4).astype(np.uint8)
    )


def make_in_maps(x: np.ndarray, W: np.ndarray) -> list[dict]:
    g4, id8, iod = _make_consts()
    wp = pack_w(W)
    in_maps = []
    for c in range(NCORES):
        in_maps.append(
            {
                "x": pack_x(x[c * BPC : (c + 1) * BPC]),
                "w": wp,
                "g4": g4,
                "id8": id8,
                "iod": iod,
            }
        )
    return in_maps


def kernel(x: np.ndarray, W: np.ndarray) -> np.ndarray:
    x = np.ascontiguousarray(np.asarray(x), dtype=np.int32)
    W = np.ascontiguousarray(np.asarray(W), dtype=np.float32)
    assert x.shape == (B, CHN, HIMG, WIMG) and W.shape == (KDIM, NOUT)

    nc = _get_nc()
    in_maps = make_in_maps(x, W)
    res = run_bass_kernel_spmd(nc, in_maps, core_ids=list(range(NCORES)))
    out = np.concatenate([res.results[c]["out"] for c in range(NCORES)], axis=0)
    return out.astype(np.float32)


if __name__ == "__main__":
    xs = np.random.randint(0, 256, (B, CHN, HIMG, WIMG)).astype(np.int32)
    Ws = (np.random.randn(KDIM, NOUT) * 0.02).astype(np.float32)
    o = kernel(xs, Ws)
    print("kernel output:", o.shape, o.dtype)



# revision 5
# speedup vs baseline: 2.7065x; 2.7065x over previous
"""Trainium2 Bass kernel: end-to-end model (pool -> linear -> max/argmax ->
top-k -> gather) distributed over 8 NeuronCores.

Strategy (v2): W is COLUMN-SHARDED across the 8 cores (38 of 304 padded
queries each) instead of replicated; x stays batch-sharded. Two small
collectives stitch it together:
  - AllGather of the pooled features (40KB/rank) so every core can compute
    its query-shard for ALL 64 samples, and
  - AllToAll of per-query results (68KB/rank) so every core receives its own
    8 samples x all 304 queries for the top-k + gather tail.
This cuts per-core W HBM traffic 8x (121MB -> 15.3MB) and PE moving-columns
14x vs the data-parallel baseline.

Pooling is done in ONE reduction per 32x32 cell from a host-side cell-major
uint8 packing, split across the Vector (tensor_reduce) and Activation
(accum_out) engines; the 1/(32*32*255) scale is folded into W on the host.

Self-contained: hardcodes all shapes; builds one SPMD Bass program and runs
it via run_bass_kernel_spmd on cores 0-7.
"""

import os
import sys
from contextlib import ExitStack

import numpy as np

for _p in ("/opt/trn_rl_repo", "/root/.axon_site/_ro/trn_rl_repo"):
    if os.path.isdir(_p) and _p not in sys.path:
        sys.path.append(_p)

import concourse.bass as bass
import concourse.tile as tile
from concourse import bacc, library_config, mybir
from concourse.bass_utils import run_bass_kernel_spmd

dt = mybir.dt
F32 = dt.float32
AX = mybir.AxisListType
OP = mybir.AluOpType

# ---------------- problem constants (hardcoded) ----------------
B, CHN, HIMG, WIMG = 64, 3, 640, 640
NQ, NCHAN, NCL, TOPK = 300, 84, 80, 150
KDIM, NOUT = 1200, 25200           # 3*20*20, NQ*NCHAN
NCORES = 8
BPC = B // NCORES                  # samples per core = 8
SCALE = np.float64(1.0) / (32 * 32 * 255)
NEG = -3.0e38
NIDX = 160                         # padded top-k index count (152 used)
NROUND = 19                        # 19 rounds x 8 = 152 >= 150

NQP = 304                          # padded query count (8 * 38)
QN = NQP // NCORES                 # queries per core = 38
KPAD = 1280                        # padded contraction dim (10 * 128)
KT = 10                            # k tiles of 128 rows
# chunk split of the 38 local queries (psum bank = 512 f32 >= 6*84)
CHQ = [6, 6, 6, 6, 6, 6, 2]
NCHUNKS = len(CHQ)
RG = [list(range(NCORES))]         # one replica group: all 8 cores

MM_DTYPE = F32  # kept for test.py's printout


def build_program():
    nc = bacc.Bacc("TRN2", target_bir_lowering=False, debug=False,
                   num_devices=NCORES)
    # x host-packed cell-major: partition p, free (b, tl, pix); cell
    # (b, k=tl*128+p) covers one 32x32 pool window, pix in [0,1024).
    x_d = nc.dram_tensor("x", [128, BPC * KT * 1024], dt.uint8,
                         kind="ExternalInput")
    # W shard host-packed per chunk-group: [128 krows, kt, cols] contiguous
    w6_d = nc.dram_tensor("w6", [6, 128, KT * CHQ[0] * NCHAN], F32,
                          kind="ExternalInput")
    w1_d = nc.dram_tensor("w1", [128, KT * CHQ[6] * NCHAN], F32,
                          kind="ExternalInput")
    iod_d = nc.dram_tensor("iod", [128, NCL], F32, kind="ExternalInput")
    out_d = nc.dram_tensor("out", [BPC, TOPK, 6], F32, kind="ExternalOutput")

    with tile.TileContext(nc) as tc:
        with ExitStack() as ctx:
            _body(ctx, tc, x_d, w6_d, w1_d, iod_d, out_d)
    nc.finalize()
    return nc


def _body(ctx, tc, x_d, w6_d, w1_d, iod_d, out_d):
    nc = tc.nc

    # ---------------- persistent tiles ----------------
    P = ctx.enter_context(tc.tile_pool(name="persist", bufs=1))

    iod = P.tile([128, NCL], F32, tag="iod")
    nc.sync.dma_start(iod[:], iod_d[:])

    s_pool = P.tile([128, BPC * KT], F32, tag="s_pool")   # raw cell sums
    pg = P.tile([128, B * KT], F32, tag="pg")             # gathered pooled
    pt = [P.tile([128, B], F32, tag=f"pt{k}", name=f"pt{k}") for k in range(KT)]
    scores = P.tile([B, QN], F32, tag="scores")           # local-query scores
    a2a_sb = P.tile([B, QN * 6 + QN], F32, tag="a2a_sb")  # interleaved + scores
    eq = P.tile([B, CHQ[0] * NCL], F32, tag="eq")
    am = P.tile([B, CHQ[0] * NCL], F32, tag="am")
    argt = P.tile([B, CHQ[0]], F32, tag="argt")
    acts = P.tile([128, 1024], F32, tag="acts")           # ACT accum dump

    feat = P.tile([128, NQP * 6], F32, tag="feat")        # gather source
    swk = P.tile([BPC, NQP], F32, tag="swk")              # topk scratch
    tv = P.tile([BPC, NROUND * 8], F32, tag="tv")
    ti = P.tile([BPC, NROUND * 8], dt.uint32, tag="ti")
    ti16 = P.tile([BPC, NIDX], dt.int16, tag="ti16")
    wrap = P.tile([128, NIDX // 16], dt.int16, tag="wrap")
    gout = P.tile([128, NIDX * 6], F32, tag="gout")

    nc.vector.memset(ti16[:, :], 0)
    nc.vector.memset(feat[:, :], 0)

    # DRAM bounce buffers for the collectives
    DP = ctx.enter_context(tc.tile_pool(name="dram", bufs=1, space="DRAM"))
    ag_in = DP.tile([128, BPC * KT], F32, tag="ag_in")
    ag_out = DP.tile([NCORES, 128, BPC * KT], F32, tag="ag_out")
    a2a_in = DP.tile([B, QN * 7], F32, tag="a2a_in")
    a2a_out = DP.tile([NCORES, BPC, QN * 7], F32, tag="a2a_out")
    tsc = DP.tile([BPC, NIDX], dt.int16, tag="tsc")

    # ---------------- phase 1: pooling (x -> s_pool [128, 80]) -------------
    # One 1024-wide sum per 32x32 cell. Per sample: DVE reduces tiles 0..4,
    # ACT accumulates tiles 5..9 -- the two engines run concurrently.
    with tc.tile_pool(name="xp", bufs=4) as XP:
        for b in range(BPC):
            xh0 = XP.tile([128, 5 * 1024], dt.uint8, tag="xh0", name="xh0")
            xh1 = XP.tile([128, 5 * 1024], dt.uint8, tag="xh1", name="xh1")
            nc.sync.dma_start(xh0[:], x_d[:, b * 10240 : b * 10240 + 5120])
            nc.scalar.dma_start(xh1[:], x_d[:, b * 10240 + 5120 : b * 10240 + 10240])
            with nc.allow_low_precision(reason="f32 sums of uint8 are exact"):
                nc.vector.tensor_reduce(
                    s_pool[:, b * KT : b * KT + 5],
                    xh0[:].rearrange("p (t x) -> p t x", x=1024),
                    axis=AX.X, op=OP.add,
                )
                for tl in range(5):
                    nc.scalar.activation(
                        acts[:],
                        xh1[:, tl * 1024 : (tl + 1) * 1024],
                        mybir.ActivationFunctionType.Copy,
                        accum_out=s_pool[:, b * KT + 5 + tl : b * KT + 6 + tl],
                    )

    # ---------------- W prefetch (issued before the AllGather bounce DMA so
    # the in-order DMA queues stream W during pooling, not after it) --------
    WP = ctx.enter_context(tc.tile_pool(name="wp", bufs=NCHUNKS))
    wts = []
    for g in range(NCHUNKS):
        cols = CHQ[g] * NCHAN
        wt = WP.tile([128, KT * cols], F32, tag="wt", name=f"wt{g}")
        eng = nc.sync if g % 2 == 0 else nc.scalar
        if g < 6:
            eng.dma_start(wt[:], w6_d[g])
        else:
            eng.dma_start(wt[:], w1_d[:])
        wts.append(wt)

    # ---------------- phase 2: AllGather pooled features -------------------
    nc.sync.dma_start(ag_in[:], s_pool[:])
    nc.gpsimd.collective_compute(
        "AllGather", OP.bypass, replica_groups=RG,
        ins=[ag_in.opt()], outs=[ag_out.opt()],
    )
    for c in range(NCORES):
        eng = nc.sync if c % 2 == 0 else nc.scalar
        eng.dma_start(pg[:, c * 80 : (c + 1) * 80], ag_out[c])
    # lhsT tiles: pt[k][p, s=(c,b)] = pooled(sample 8c+b, krow k*128+p)
    pgv = pg[:].rearrange("p (c b t) -> p t c b", c=NCORES, b=BPC)
    for k in range(KT):
        nc.vector.tensor_copy(
            pt[k][:].rearrange("p (c b) -> p c b", c=NCORES), pgv[:, k]
        )

    # ---------------- phase 3: sharded matmul + per-chunk postproc ---------
    a2v = a2a_sb[:, : QN * 6].rearrange("b (q c) -> b q c", c=6)
    with tc.tile_pool(name="yps", bufs=6, space="PSUM") as YPS:
        q0 = 0
        for g in range(NCHUNKS):
            nq = CHQ[g]
            cols = nq * NCHAN
            psy = YPS.tile([B, cols], F32, tag="psy", name="psy")
            for k in range(KT):
                nc.tensor.matmul(
                    psy[:], pt[k][:], wts[g][:, k * cols : (k + 1) * cols],
                    start=(k == 0), stop=(k == KT - 1),
                )
            psv = psy[:].rearrange("b (q c) -> b q c", c=NCHAN)
            # boxes straight into the interleaved AllToAll layout
            nc.vector.tensor_copy(a2v[:, q0 : q0 + nq, 0:4], psv[:, :, 0:4])
            # per-query max score
            nc.vector.tensor_reduce(
                scores[:, q0 : q0 + nq], psv[:, :, 4:NCHAN], axis=AX.X, op=OP.max
            )
            # argmax over classes: first-index ties via iod = 79 - class_id
            eqv = eq[:, : nq * NCL].rearrange("b (q c) -> b q c", c=NCL)
            nc.vector.tensor_tensor(
                eqv, psv[:, :, 4:NCHAN],
                scores[:, q0 : q0 + nq].unsqueeze(-1).broadcast_to((B, nq, NCL)),
                op=OP.is_ge,
            )
            amv = am[:, : nq * NCL].rearrange("b (q c) -> b q c", c=NCL)
            nc.vector.tensor_tensor(
                amv, eqv,
                iod[:B, :].unsqueeze(1).broadcast_to((B, nq, NCL)),
                op=OP.mult,
            )
            nc.vector.tensor_reduce(argt[:, :nq], amv, axis=AX.X, op=OP.max)
            nc.vector.tensor_scalar(
                a2v[:, q0 : q0 + nq, 5], argt[:, :nq], -1.0, float(NCL - 1),
                op0=OP.mult, op1=OP.add,
            )
            q0 += nq

    # ---------------- phase 4: AllToAll per-query results ------------------
    nc.vector.tensor_copy(a2v[:, :, 4], scores[:])
    nc.vector.tensor_copy(a2a_sb[:, QN * 6 :], scores[:])
    nc.sync.dma_start(a2a_in[:], a2a_sb[:])
    nc.gpsimd.collective_compute(
        "AllToAll", OP.bypass, replica_groups=RG,
        ins=[a2a_in.opt()], outs=[a2a_out.opt()],
    )
    nc.gpsimd.load_library(library_config.ap_gather)

    # ---------------- phase 5: top-150 tail --------------------------------
    # feat[16b] = sample b's [304, 6] rows (concat of the 8 cores' blocks)
    for b in range(BPC):
        eng = nc.sync if b % 2 == 0 else nc.scalar
        eng.dma_start(
            feat[16 * b : 16 * b + 1, :].rearrange("o (c x) -> o c x", c=NCORES),
            a2a_out[:, b, : QN * 6].unsqueeze(0),
        )
    # swk[b, c*38+q] = score of global query c*38+q for sample b
    nc.scalar.dma_start(
        swk[:].rearrange("b (c q) -> b c q", c=NCORES),
        a2a_out[:, :, QN * 6 :].rearrange("c b q -> b c q"),
    )
    nc.vector.memset(swk[:, NQ:NQP], NEG)  # padded queries never win

    for r in range(NROUND):
        nc.vector.max(tv[:, 8 * r : 8 * r + 8], swk[:, :])
        nc.vector.max_index(ti[:, 8 * r : 8 * r + 8], tv[:, 8 * r : 8 * r + 8], swk[:, :])
        if r < NROUND - 1:
            nc.vector.match_replace(
                swk[:, :], tv[:, 8 * r : 8 * r + 8], swk[:, :], NEG
            )
    nc.vector.tensor_copy(ti16[:, : NROUND * 8], ti[:, :])

    # wrap indices into per-core [16, 10] layout (via DRAM)
    nc.scalar.dma_start(tsc[:], ti16[:])
    for b in range(BPC):
        eng = nc.sync if b % 2 == 0 else nc.scalar
        eng.dma_start(
            wrap[16 * b : 16 * b + 16, :],
            tsc[b].rearrange("(f p) -> p f", p=16),
        )

    nc.gpsimd.ap_gather(
        gout[:].rearrange("p (i c) -> p i c", c=6),
        feat[:].rearrange("p (q c) -> p q c", c=6),
        wrap[:],
        channels=128,
        num_elems=NQP,
        d=6,
        num_idxs=NIDX,
    )
    for b in range(BPC):
        eng = nc.sync if b % 2 == 0 else nc.scalar
        eng.dma_start(
            out_d[b : b + 1].rearrange("o k c -> o (k c)"),
            gout[16 * b : 16 * b + 1, : TOPK * 6],
        )


def _make_consts():
    iod = np.broadcast_to(
        (np.float32(NCL - 1) - np.arange(NCL, dtype=np.float32))[None, :], (128, NCL)
    ).copy()
    return iod


_NC_CACHE = {}


def _get_nc():
    if "nc" not in _NC_CACHE:
        _NC_CACHE["nc"] = build_program()
    return _NC_CACHE["nc"]


def pack_x(xs: np.ndarray) -> np.ndarray:
    """[BPC, 3, 640, 640] int32 -> [128, BPC*10*1024] uint8 cell-major.

    Cell k = c_rgb*400 + i*20 + j (matching W's row layout after the
    BGR->RGB flip); cell (b, k) sits at partition k%128, free offset
    b*10240 + (k//128)*1024; cells 1200..1279 are zero padding.
    """
    xs8 = xs.astype(np.uint8).reshape(BPC, CHN, 20, 32, 20, 32)
    xs8 = xs8[:, ::-1]  # BGR -> RGB
    cells = xs8.transpose(0, 1, 2, 4, 3, 5).reshape(BPC, KDIM, 1024)
    full = np.zeros((BPC, KPAD, 1024), np.uint8)
    full[:, :KDIM] = cells
    # [b, tl, p, pix] -> [p, b, tl, pix]
    return np.ascontiguousarray(
        full.reshape(BPC, KT, 128, 1024).transpose(2, 0, 1, 3)
    ).reshape(128, BPC * KT * 1024)


def pack_w(W: np.ndarray) -> tuple[np.ndarray, np.ndarray]:
    """[1200, 25200] -> per-core chunk-group tiles (scale folded in).

    Returns (w6 [8, 6, 128, 5040], w1 [8, 128, 1680]): core c, group g holds
    [128 krows, kt, cols] for its query columns, kpad rows 1200..1279 zero.
    """
    Wp = np.zeros((KPAD, NQP * NCHAN), np.float32)
    Wp[:KDIM, : NQ * NCHAN] = (W.astype(np.float64) * SCALE).astype(np.float32)
    w6 = np.zeros((NCORES, 6, 128, KT * CHQ[0] * NCHAN), np.float32)
    w1 = np.zeros((NCORES, 128, KT * CHQ[6] * NCHAN), np.float32)
    for c in range(NCORES):
        s = Wp[:, c * QN * NCHAN : (c + 1) * QN * NCHAN]
        q0 = 0
        for g in range(NCHUNKS):
            cols = CHQ[g] * NCHAN
            blk = s[:, q0 : q0 + cols].reshape(KT, 128, cols).transpose(1, 0, 2)
            if g < 6:
                w6[c, g] = blk.reshape(128, KT * cols)
            else:
                w1[c] = blk.reshape(128, KT * cols)
            q0 += cols
    return w6, w1


def make_in_maps(x: np.ndarray, W: np.ndarray) -> list[dict]:
    iod = _make_consts()
    w6, w1 = pack_w(W)
    in_maps = []
    for c in range(NCORES):
        in_maps.append(
            {
                "x": pack_x(x[c * BPC : (c + 1) * BPC]),
                "w6": w6[c],
                "w1": w1[c],
                "iod": iod,
            }
        )
    return in_maps


def kernel(x: np.ndarray, W: np.ndarray) -> np.ndarray:
    x = np.ascontiguousarray(np.asarray(x), dtype=np.int32)
    W = np.ascontiguousarray(np.asarray(W), dtype=np.float32)
    assert x.shape == (B, CHN, HIMG, WIMG) and W.shape == (KDIM, NOUT)

    nc = _get_nc()
    in_maps = make_in_maps(x, W)
    res = run_bass_kernel_spmd(nc, in_maps, core_ids=list(range(NCORES)))
    out = np.concatenate([res.results[c]["out"] for c in range(NCORES)], axis=0)
    return out.astype(np.float32)


if __name__ == "__main__":
    xs = np.random.randint(0, 256, (B, CHN, HIMG, WIMG)).astype(np.int32)
    Ws = (np.random.randn(KDIM, NOUT) * 0.02).astype(np.float32)
    o = kernel(xs, Ws)
    print("kernel output:", o.shape, o.dtype)


# revision 17
# speedup vs baseline: 2.9628x; 1.0947x over previous
"""Trainium2 Bass kernel: end-to-end model (pool -> linear -> max/argmax ->
top-k -> gather) distributed over 8 NeuronCores.

Strategy (v2): W is COLUMN-SHARDED across the 8 cores (38 of 304 padded
queries each) instead of replicated; x stays batch-sharded. Two small
collectives stitch it together:
  - AllGather of the pooled features (40KB/rank) so every core can compute
    its query-shard for ALL 64 samples, and
  - AllToAll of per-query results (68KB/rank) so every core receives its own
    8 samples x all 304 queries for the top-k + gather tail.
This cuts per-core W HBM traffic 8x (121MB -> 15.3MB) and PE moving-columns
14x vs the data-parallel baseline.

Pooling is done in ONE reduction per 32x32 cell from a host-side cell-major
uint8 packing, split across the Vector (tensor_reduce) and Activation
(accum_out) engines; the 1/(32*32*255) scale is folded into W on the host.

Self-contained: hardcodes all shapes; builds one SPMD Bass program and runs
it via run_bass_kernel_spmd on cores 0-7.
"""

import os
import sys
from contextlib import ExitStack

import numpy as np

for _p in ("/opt/trn_rl_repo", "/root/.axon_site/_ro/trn_rl_repo"):
    if os.path.isdir(_p) and _p not in sys.path:
        sys.path.append(_p)

import concourse.bass as bass
import concourse.tile as tile
from concourse import bacc, library_config, mybir
from concourse.bass_utils import run_bass_kernel_spmd

dt = mybir.dt
F32 = dt.float32
AX = mybir.AxisListType
OP = mybir.AluOpType

# ---------------- problem constants (hardcoded) ----------------
B, CHN, HIMG, WIMG = 64, 3, 640, 640
NQ, NCHAN, NCL, TOPK = 300, 84, 80, 150
KDIM, NOUT = 1200, 25200           # 3*20*20, NQ*NCHAN
NCORES = 8
BPC = B // NCORES                  # samples per core = 8
SCALE = np.float64(1.0) / (32 * 32 * 255)
NEG = -3.0e38
NIDX = 160                         # padded top-k index count (152 used)
NROUND = 19                        # 19 rounds x 8 = 152 >= 150

NQP = 304                          # padded query count (8 * 38)
QN = NQP // NCORES                 # queries per core = 38
KPAD = 1280                        # padded contraction dim (10 * 128)
KT = 10                            # k tiles of 128 rows
# chunk split of the 38 local queries (psum bank = 512 f32 >= 6*84)
CHQ = [6, 6, 6, 6, 6, 6, 2]
NCHUNKS = len(CHQ)
RG = [list(range(NCORES))]         # one replica group: all 8 cores

MM_DTYPE = F32  # kept for test.py's printout


def build_program():
    nc = bacc.Bacc("TRN2", target_bir_lowering=False, debug=False,
                   num_devices=NCORES)
    # x host-packed cell-major: partition p, free (b, tl, pix); cell
    # (b, k=tl*128+p) covers one 32x32 pool window, pix in [0,1024).
    x_d = nc.dram_tensor("x", [128, BPC * KT * 1024], dt.uint8,
                         kind="ExternalInput")
    # W shard host-packed per chunk-group: [128 krows, kt, cols] contiguous
    w6_d = nc.dram_tensor("w6", [6, 128, KT * CHQ[0] * NCHAN], F32,
                          kind="ExternalInput")
    w1_d = nc.dram_tensor("w1", [128, KT * CHQ[6] * NCHAN], F32,
                          kind="ExternalInput")
    iod_d = nc.dram_tensor("iod", [128, NCL], F32, kind="ExternalInput")
    out_d = nc.dram_tensor("out", [BPC, TOPK, 6], F32, kind="ExternalOutput")
    if os.environ.get("KERNEL_DEBUG", "0") == "1":
        dbg = {
            "dti16": nc.dram_tensor("dti16", [BPC, NIDX], dt.int16, kind="ExternalOutput"),
            "dwrap": nc.dram_tensor("dwrap", [128, NIDX // 16], dt.int16, kind="ExternalOutput"),
            "dtv": nc.dram_tensor("dtv", [BPC, NROUND * 8], F32, kind="ExternalOutput"),
            "dgout": nc.dram_tensor("dgout", [128, NIDX * 6], F32, kind="ExternalOutput"),
        }
    else:
        dbg = None

    with tile.TileContext(nc) as tc:
        with ExitStack() as ctx:
            _body(ctx, tc, x_d, w6_d, w1_d, iod_d, out_d, dbg)
    nc.finalize()
    return nc


def _body(ctx, tc, x_d, w6_d, w1_d, iod_d, out_d, dbg=None):
    nc = tc.nc

    # ---------------- persistent tiles ----------------
    P = ctx.enter_context(tc.tile_pool(name="persist", bufs=1))

    iod = P.tile([128, NCL], F32, tag="iod")
    nc.sync.dma_start(iod[:], iod_d[:])

    s_pool = P.tile([128, BPC * KT], F32, tag="s_pool")   # raw cell sums
    pg = P.tile([128, B * KT], F32, tag="pg")             # gathered pooled
    scores = P.tile([B, QN], F32, tag="scores")           # local-query scores
    a2a_sb = P.tile([B, QN * 6 + QN], F32, tag="a2a_sb")  # interleaved + scores
    eq = P.tile([B, CHQ[0] * NCL], F32, tag="eq")
    am = P.tile([B, CHQ[0] * NCL], F32, tag="am")
    argt = P.tile([B, CHQ[0]], F32, tag="argt")
    acts = P.tile([128, 1024], F32, tag="acts")           # ACT accum dump

    feat = P.tile([128, NQP * 6], F32, tag="feat")        # gather source
    swk = P.tile([BPC, NQP], F32, tag="swk")              # topk scratch
    tv = P.tile([BPC, NROUND * 8], F32, tag="tv")
    ti = P.tile([BPC, NROUND * 8], dt.uint32, tag="ti")
    ti16 = P.tile([BPC, NIDX], dt.int16, tag="ti16")
    wraps = [P.tile([128, 5], dt.int16, tag=f"wrap{h}", name=f"wrap{h}")
             for h in range(2)]
    gout = P.tile([128, NIDX * 6], F32, tag="gout")

    # DRAM bounce buffers for the collectives
    DP = ctx.enter_context(tc.tile_pool(name="dram", bufs=1, space="DRAM"))
    ag_in = DP.tile([128, BPC * KT], F32, tag="ag_in")
    ag_out = DP.tile([NCORES, 128, BPC * KT], F32, tag="ag_out")
    a2a_in = DP.tile([B, QN * 7], F32, tag="a2a_in")
    a2a_out = DP.tile([NCORES, BPC, QN * 7], F32, tag="a2a_out")
    tsc = DP.tile([BPC, NIDX], dt.int16, tag="tsc")
    warm_in = DP.tile([1, 8], F32, tag="warm_in")
    warm_out = DP.tile([NCORES, 8], F32, tag="warm_out")

    # Fire a tiny dummy collective immediately: the FIRST collective pays
    # ~12us of ncfw cold-start; absorb it during pooling. Also load the
    # gpsimd gather library up front (its drain overlaps pooling too).
    nc.scalar.dma_start(warm_in[:], iod[0:1, 0:8])
    nc.gpsimd.collective_compute(
        "AllGather", OP.bypass, replica_groups=RG,
        ins=[warm_in.opt()], outs=[warm_out.opt()],
    )
    nc.gpsimd.load_library(library_config.ap_gather)

    # ---------------- phase 1: pooling (x -> s_pool [128, 80]) -------------
    # One 1024-wide sum per 32x32 cell. Per sample: DVE reduces the first
    # ndv tiles, ACT accumulates the rest -- the two engines run
    # concurrently. ACT is slightly slower per tile and also pays the
    # act-table load, so the last samples shift one tile to DVE.
    with tc.tile_pool(name="xp", bufs=4) as XP:
        for b in range(BPC):
            ndv = 5 if b < 6 else 6
            xh0 = XP.tile([128, 6 * 1024], dt.uint8, tag="xh0", name="xh0")
            xh1 = XP.tile([128, 5 * 1024], dt.uint8, tag="xh1", name="xh1")
            nc.sync.dma_start(
                xh0[:, : ndv * 1024], x_d[:, b * 10240 : b * 10240 + ndv * 1024]
            )
            nc.scalar.dma_start(
                xh1[:, : (KT - ndv) * 1024],
                x_d[:, b * 10240 + ndv * 1024 : (b + 1) * 10240],
            )
            with nc.allow_low_precision(reason="f32 sums of uint8 are exact"):
                nc.vector.tensor_reduce(
                    s_pool[:, b * KT : b * KT + ndv],
                    xh0[:, : ndv * 1024].rearrange("p (t x) -> p t x", x=1024),
                    axis=AX.X, op=OP.add,
                )
                for tl in range(ndv, KT):
                    nc.scalar.activation(
                        acts[:],
                        xh1[:, (tl - ndv) * 1024 : (tl - ndv + 1) * 1024],
                        mybir.ActivationFunctionType.Copy,
                        accum_out=s_pool[:, b * KT + tl : b * KT + tl + 1],
                    )
    nc.vector.memset(ti16[:, :], 0)
    nc.vector.memset(feat[:, :], 0)

    # ---------------- W prefetch (issued before the AllGather bounce DMA so
    # the in-order DMA queues stream W during pooling, not after it) --------
    WP = ctx.enter_context(tc.tile_pool(name="wp", bufs=NCHUNKS))
    wts = []
    for g in range(NCHUNKS):
        cols = CHQ[g] * NCHAN
        wt = WP.tile([128, KT * cols], F32, tag="wt", name=f"wt{g}")
        if g < 6:
            nc.sync.dma_start(wt[:], w6_d[g])
        else:
            nc.sync.dma_start(wt[:], w1_d[:])
        wts.append(wt)

    # ---------------- phase 2: AllGather pooled features -------------------
    # Bounce DMAs ride the (idle) TensorE queue: the sync/scalar HWDGE
    # queues are still draining the bulk x/W streams, and queues execute
    # in order -- a critical 40KB DMA must not sit behind megabytes of W.
    nc.scalar.dma_start(ag_in[:], s_pool[:])
    nc.gpsimd.collective_compute(
        "AllGather", OP.bypass, replica_groups=RG,
        ins=[ag_in.opt()], outs=[ag_out.opt()],
    )
    nc.scalar.dma_start(
        pg[:].rearrange("p (c t) -> p c t", c=NCORES),
        ag_out[:].rearrange("c p t -> p c t"),
    )
    # lhsT tiles: pt_all[p, (t, s=(c,b))] = pooled(sample 8c+b, krow t*128+p)
    pt_all = P.tile([128, KT * B], F32, tag="pt_all")
    nc.vector.tensor_copy(
        pt_all[:].rearrange("p (t c b) -> p t c b", t=KT, c=NCORES),
        pg[:].rearrange("p (c b t) -> p t c b", c=NCORES, b=BPC),
    )

    # ---------------- phase 3: sharded matmul + per-chunk postproc ---------
    a2v = a2a_sb[:, : QN * 6].rearrange("b (q c) -> b q c", c=6)
    with tc.tile_pool(name="yps", bufs=6, space="PSUM") as YPS:
        q0 = 0
        for g in range(NCHUNKS):
            nq = CHQ[g]
            cols = nq * NCHAN
            psy = YPS.tile([B, cols], F32, tag="psy", name="psy")
            for k in range(KT):
                nc.tensor.matmul(
                    psy[:], pt_all[:, k * B : (k + 1) * B],
                    wts[g][:, k * cols : (k + 1) * cols],
                    start=(k == 0), stop=(k == KT - 1),
                )
            psv = psy[:].rearrange("b (q c) -> b q c", c=NCHAN)
            # boxes straight into the interleaved AllToAll layout
            nc.vector.tensor_copy(a2v[:, q0 : q0 + nq, 0:4], psv[:, :, 0:4])
            # per-query max score
            nc.vector.tensor_reduce(
                scores[:, q0 : q0 + nq], psv[:, :, 4:NCHAN], axis=AX.X, op=OP.max
            )
            # argmax over classes: first-index ties via iod = 79 - class_id
            eqv = eq[:, : nq * NCL].rearrange("b (q c) -> b q c", c=NCL)
            nc.vector.tensor_tensor(
                eqv, psv[:, :, 4:NCHAN],
                scores[:, q0 : q0 + nq].unsqueeze(-1).broadcast_to((B, nq, NCL)),
                op=OP.is_ge,
            )
            amv = am[:, : nq * NCL].rearrange("b (q c) -> b q c", c=NCL)
            nc.vector.tensor_tensor(
                amv, eqv,
                iod[:B, :].unsqueeze(1).broadcast_to((B, nq, NCL)),
                op=OP.mult,
            )
            nc.vector.tensor_reduce(argt[:, :nq], amv, axis=AX.X, op=OP.max)
            nc.vector.tensor_scalar(
                a2v[:, q0 : q0 + nq, 5], argt[:, :nq], -1.0, float(NCL - 1),
                op0=OP.mult, op1=OP.add,
            )
            q0 += nq

    # ---------------- phase 4: AllToAll per-query results ------------------
    nc.vector.tensor_copy(a2v[:, :, 4], scores[:])
    nc.vector.tensor_copy(a2a_sb[:, QN * 6 :], scores[:])
    nc.scalar.dma_start(a2a_in[:], a2a_sb[:])
    nc.gpsimd.collective_compute(
        "AllToAll", OP.bypass, replica_groups=RG,
        ins=[a2a_in.opt()], outs=[a2a_out.opt()],
    )

    # ---------------- phase 5: top-150 tail --------------------------------
    # feat[16b] = sample b's [304, 6] rows (concat of the 8 cores' blocks)
    nc.sync.dma_start(
        feat[:].rearrange("(b s) (c x) -> b s c x", b=BPC, c=NCORES)[:, 0],
        a2a_out[:, :, : QN * 6].rearrange("c b x -> b c x"),
    )
    # swk[b, c*38+q] = score of global query c*38+q for sample b
    nc.scalar.dma_start(
        swk[:].rearrange("b (c q) -> b c q", c=NCORES),
        a2a_out[:, :, QN * 6 :].rearrange("c b q -> b c q"),
    )
    nc.vector.memset(swk[:, NQ:NQP], NEG)  # padded queries never win

    # two-half tail: indices from rounds 0..9 are wrapped + gathered while
    # rounds 10..18 still run on DVE
    def wrap_and_gather(h):
        i0, i1 = h * 80, (h + 1) * 80
        nc.vector.tensor_copy(ti16[:, i0 : min(i1, NROUND * 8)], ti[:, i0 : min(i1, NROUND * 8)])
        nc.scalar.dma_start(tsc[:, i0:i1], ti16[:, i0:i1])
        for b in range(BPC):
            eng = nc.sync if b % 2 == 0 else nc.scalar
            eng.dma_start(
                wraps[h][16 * b : 16 * b + 16, :],
                tsc[b, i0:i1].rearrange("(f p) -> p f", p=16),
            )
        nc.gpsimd.ap_gather(
            gout[:].rearrange("p (i c) -> p i c", c=6)[:, i0:i1],
            feat[:].rearrange("p (q c) -> p q c", c=6),
            wraps[h][:],
            channels=128,
            num_elems=NQP,
            d=6,
            num_idxs=80,
        )

    for r in range(NROUND):
        nc.vector.max(tv[:, 8 * r : 8 * r + 8], swk[:, :])
        nc.vector.max_index(ti[:, 8 * r : 8 * r + 8], tv[:, 8 * r : 8 * r + 8], swk[:, :])
        if r < NROUND - 1:
            nc.vector.match_replace(
                swk[:, :], tv[:, 8 * r : 8 * r + 8], swk[:, :], NEG
            )
        if r == 9:
            wrap_and_gather(0)
    wrap_and_gather(1)

    nc.scalar.dma_start(
        out_d[:].rearrange("b k c -> b (k c)"),
        gout[:].rearrange("(b s) x -> b s x", b=BPC)[:, 0, : TOPK * 6],
    )
    if dbg is not None:
        nc.sync.dma_start(dbg["dti16"][:], ti16[:])
        nc.sync.dma_start(dbg["dwrap"][:, 0:5], wraps[0][:])
        nc.sync.dma_start(dbg["dwrap"][:, 5:10], wraps[1][:])
        nc.sync.dma_start(dbg["dtv"][:], tv[:])
        nc.sync.dma_start(dbg["dgout"][:], gout[:])


def _make_consts():
    iod = np.broadcast_to(
        (np.float32(NCL - 1) - np.arange(NCL, dtype=np.float32))[None, :], (128, NCL)
    ).copy()
    return iod


_NC_CACHE = {}


def _get_nc():
    if "nc" not in _NC_CACHE:
        _NC_CACHE["nc"] = build_program()
    return _NC_CACHE["nc"]


def pack_x(xs: np.ndarray) -> np.ndarray:
    """[BPC, 3, 640, 640] int32 -> [128, BPC*10*1024] uint8 cell-major.

    Cell k = c_rgb*400 + i*20 + j (matching W's row layout after the
    BGR->RGB flip); cell (b, k) sits at partition k%128, free offset
    b*10240 + (k//128)*1024; cells 1200..1279 are zero padding.
    """
    xs8 = xs.astype(np.uint8).reshape(BPC, CHN, 20, 32, 20, 32)
    xs8 = xs8[:, ::-1]  # BGR -> RGB
    cells = xs8.transpose(0, 1, 2, 4, 3, 5).reshape(BPC, KDIM, 1024)
    full = np.zeros((BPC, KPAD, 1024), np.uint8)
    full[:, :KDIM] = cells
    # [b, tl, p, pix] -> [p, b, tl, pix]
    return np.ascontiguousarray(
        full.reshape(BPC, KT, 128, 1024).transpose(2, 0, 1, 3)
    ).reshape(128, BPC * KT * 1024)


def pack_w(W: np.ndarray) -> tuple[np.ndarray, np.ndarray]:
    """[1200, 25200] -> per-core chunk-group tiles (scale folded in).

    Returns (w6 [8, 6, 128, 5040], w1 [8, 128, 1680]): core c, group g holds
    [128 krows, kt, cols] for its query columns, kpad rows 1200..1279 zero.
    """
    Wp = np.zeros((KPAD, NQP * NCHAN), np.float32)
    Wp[:KDIM, : NQ * NCHAN] = (W.astype(np.float64) * SCALE).astype(np.float32)
    w6 = np.zeros((NCORES, 6, 128, KT * CHQ[0] * NCHAN), np.float32)
    w1 = np.zeros((NCORES, 128, KT * CHQ[6] * NCHAN), np.float32)
    for c in range(NCORES):
        s = Wp[:, c * QN * NCHAN : (c + 1) * QN * NCHAN]
        q0 = 0
        for g in range(NCHUNKS):
            cols = CHQ[g] * NCHAN
            blk = s[:, q0 : q0 + cols].reshape(KT, 128, cols).transpose(1, 0, 2)
            if g < 6:
                w6[c, g] = blk.reshape(128, KT * cols)
            else:
                w1[c] = blk.reshape(128, KT * cols)
            q0 += cols
    return w6, w1


def make_in_maps(x: np.ndarray, W: np.ndarray) -> list[dict]:
    iod = _make_consts()
    w6, w1 = pack_w(W)
    in_maps = []
    for c in range(NCORES):
        in_maps.append(
            {
                "x": pack_x(x[c * BPC : (c + 1) * BPC]),
                "w6": w6[c],
                "w1": w1[c],
                "iod": iod,
            }
        )
    return in_maps


def kernel(x: np.ndarray, W: np.ndarray) -> np.ndarray:
    x = np.ascontiguousarray(np.asarray(x), dtype=np.int32)
    W = np.ascontiguousarray(np.asarray(W), dtype=np.float32)
    assert x.shape == (B, CHN, HIMG, WIMG) and W.shape == (KDIM, NOUT)

    nc = _get_nc()
    in_maps = make_in_maps(x, W)
    res = run_bass_kernel_spmd(nc, in_maps, core_ids=list(range(NCORES)))
    out = np.concatenate([res.results[c]["out"] for c in range(NCORES)], axis=0)
    return out.astype(np.float32)


if __name__ == "__main__":
    xs = np.random.randint(0, 256, (B, CHN, HIMG, WIMG)).astype(np.int32)
    Ws = (np.random.randn(KDIM, NOUT) * 0.02).astype(np.float32)
    o = kernel(xs, Ws)
    print("kernel output:", o.shape, o.dtype)


# revision 19
# speedup vs baseline: 3.0884x; 1.0424x over previous
"""Trainium2 Bass kernel: end-to-end model (pool -> linear -> max/argmax ->
top-k -> gather) distributed over 8 NeuronCores.

Strategy (v2): W is COLUMN-SHARDED across the 8 cores (38 of 304 padded
queries each) instead of replicated; x stays batch-sharded. Two small
collectives stitch it together:
  - AllGather of the pooled features (40KB/rank) so every core can compute
    its query-shard for ALL 64 samples, and
  - AllToAll of per-query results (68KB/rank) so every core receives its own
    8 samples x all 304 queries for the top-k + gather tail.
This cuts per-core W HBM traffic 8x (121MB -> 15.3MB) and PE moving-columns
14x vs the data-parallel baseline.

Pooling is done in ONE reduction per 32x32 cell from a host-side cell-major
uint8 packing, split across the Vector (tensor_reduce) and Activation
(accum_out) engines; the 1/(32*32*255) scale is folded into W on the host.

Self-contained: hardcodes all shapes; builds one SPMD Bass program and runs
it via run_bass_kernel_spmd on cores 0-7.
"""

import os
import sys
from contextlib import ExitStack

import numpy as np

for _p in ("/opt/trn_rl_repo", "/root/.axon_site/_ro/trn_rl_repo"):
    if os.path.isdir(_p) and _p not in sys.path:
        sys.path.append(_p)

import concourse.bass as bass
import concourse.tile as tile
from concourse import bacc, library_config, mybir
from concourse.bass_utils import run_bass_kernel_spmd

dt = mybir.dt
F32 = dt.float32
AX = mybir.AxisListType
OP = mybir.AluOpType

# ---------------- problem constants (hardcoded) ----------------
B, CHN, HIMG, WIMG = 64, 3, 640, 640
NQ, NCHAN, NCL, TOPK = 300, 84, 80, 150
KDIM, NOUT = 1200, 25200           # 3*20*20, NQ*NCHAN
NCORES = 8
BPC = B // NCORES                  # samples per core = 8
SCALE = np.float64(1.0) / (32 * 32 * 255)
NEG = -3.0e38
NIDX = 160                         # padded top-k index count (152 used)
NROUND = 19                        # 19 rounds x 8 = 152 >= 150

NQP = 304                          # padded query count (8 * 38)
QN = NQP // NCORES                 # queries per core = 38
KPAD = 1280                        # padded contraction dim (10 * 128)
KT = 10                            # k tiles of 128 rows
# chunk split of the 38 local queries (psum bank = 512 f32 >= 6*84)
CHQ = [6, 6, 6, 6, 6, 6, 2]
NCHUNKS = len(CHQ)
RG = [list(range(NCORES))]         # one replica group: all 8 cores

MM_DTYPE = F32  # kept for test.py's printout


def build_program():
    nc = bacc.Bacc("TRN2", target_bir_lowering=False, debug=False,
                   num_devices=NCORES)
    # x host-packed cell-major: partition p, free (b, tl, pix); cell
    # (b, k=tl*128+p) covers one 32x32 pool window, pix in [0,1024).
    x_d = nc.dram_tensor("x", [128, BPC * KT * 1024], dt.uint8,
                         kind="ExternalInput")
    # W shard host-packed per chunk-group: [128 krows, kt, cols] contiguous
    w6_d = nc.dram_tensor("w6", [6, 128, KT * CHQ[0] * NCHAN], F32,
                          kind="ExternalInput")
    w1_d = nc.dram_tensor("w1", [128, KT * CHQ[6] * NCHAN], F32,
                          kind="ExternalInput")
    iod_d = nc.dram_tensor("iod", [128, NCL], F32, kind="ExternalInput")
    out_d = nc.dram_tensor("out", [BPC, TOPK, 6], F32, kind="ExternalOutput")
    if os.environ.get("KERNEL_DEBUG", "0") == "1":
        dbg = {
            "dti16": nc.dram_tensor("dti16", [BPC, NIDX], dt.int16, kind="ExternalOutput"),
            "dwrap": nc.dram_tensor("dwrap", [128, NIDX // 16], dt.int16, kind="ExternalOutput"),
            "dtv": nc.dram_tensor("dtv", [BPC, NROUND * 8], F32, kind="ExternalOutput"),
            "dgout": nc.dram_tensor("dgout", [128, NIDX * 6], F32, kind="ExternalOutput"),
        }
    else:
        dbg = None

    with tile.TileContext(nc) as tc:
        with ExitStack() as ctx:
            _body(ctx, tc, x_d, w6_d, w1_d, iod_d, out_d, dbg)
    nc.finalize()
    return nc


def _body(ctx, tc, x_d, w6_d, w1_d, iod_d, out_d, dbg=None):
    nc = tc.nc

    # ---------------- persistent tiles ----------------
    P = ctx.enter_context(tc.tile_pool(name="persist", bufs=1))

    iod = P.tile([128, NCL], F32, tag="iod")
    nc.sync.dma_start(iod[:], iod_d[:])

    s_pool = P.tile([128, BPC * KT], F32, tag="s_pool")   # raw cell sums
    pg = P.tile([128, B * KT], F32, tag="pg")             # gathered pooled
    scores = P.tile([B, QN], F32, tag="scores")           # local-query scores
    a2a_sb = P.tile([B, QN * 6 + QN], F32, tag="a2a_sb")  # interleaved + scores
    eq = P.tile([B, CHQ[0] * NCL], F32, tag="eq")
    am = P.tile([B, CHQ[0] * NCL], F32, tag="am")
    argt = P.tile([B, CHQ[0]], F32, tag="argt")
    acts = P.tile([128, 1024], F32, tag="acts")           # ACT accum dump

    feat = P.tile([128, NQP * 6], F32, tag="feat")        # gather source
    swk = P.tile([BPC, NQP], F32, tag="swk")              # topk scratch
    tv = P.tile([BPC, NROUND * 8], F32, tag="tv")
    ti = P.tile([BPC, NROUND * 8], dt.uint32, tag="ti")
    ti16 = P.tile([BPC, NIDX], dt.int16, tag="ti16")
    wraps = [P.tile([128, 5], dt.int16, tag=f"wrap{h}", name=f"wrap{h}")
             for h in range(2)]
    gout = P.tile([128, NIDX * 6], F32, tag="gout")

    # DRAM bounce buffers for the collectives
    DP = ctx.enter_context(tc.tile_pool(name="dram", bufs=1, space="DRAM"))
    ag_in = DP.tile([128, BPC * KT], F32, tag="ag_in")
    ag_out = DP.tile([NCORES, 128, BPC * KT], F32, tag="ag_out")
    a2a_in = DP.tile([B, QN * 7], F32, tag="a2a_in")
    a2a_out = DP.tile([NCORES, BPC, QN * 7], F32, tag="a2a_out")
    tsc = DP.tile([BPC, NIDX], dt.int16, tag="tsc")

    # load the gpsimd gather library up front (its drain overlaps pooling)
    nc.gpsimd.load_library(library_config.ap_gather)

    # ---------------- phase 1: pooling (x -> s_pool [128, 80]) -------------
    # One 1024-wide sum per 32x32 cell. Per sample: DVE reduces the first
    # ndv tiles, ACT accumulates the rest -- the two engines run
    # concurrently. ACT is slightly slower per tile and also pays the
    # act-table load, so the last samples shift one tile to DVE.
    with tc.tile_pool(name="xp", bufs=4) as XP:
        for b in range(BPC):
            ndv = 6 if b < 6 else 5
            xh0 = XP.tile([128, 6 * 1024], dt.uint8, tag="xh0", name="xh0")
            xh1 = XP.tile([128, 5 * 1024], dt.uint8, tag="xh1", name="xh1")
            nc.sync.dma_start(
                xh0[:, : ndv * 1024], x_d[:, b * 10240 : b * 10240 + ndv * 1024]
            )
            nc.scalar.dma_start(
                xh1[:, : (KT - ndv) * 1024],
                x_d[:, b * 10240 + ndv * 1024 : (b + 1) * 10240],
            )
            with nc.allow_low_precision(reason="f32 sums of uint8 are exact"):
                nc.vector.tensor_reduce(
                    s_pool[:, b * KT : b * KT + ndv],
                    xh0[:, : ndv * 1024].rearrange("p (t x) -> p t x", x=1024),
                    axis=AX.X, op=OP.add,
                )
                for tl in range(ndv, KT):
                    nc.scalar.activation(
                        acts[:],
                        xh1[:, (tl - ndv) * 1024 : (tl - ndv + 1) * 1024],
                        mybir.ActivationFunctionType.Copy,
                        accum_out=s_pool[:, b * KT + tl : b * KT + tl + 1],
                    )
    nc.gpsimd.memset(ti16[:, :], 0)
    nc.gpsimd.memset(feat[:, :], 0)

    # ---------------- W prefetch (issued before the AllGather bounce DMA so
    # the in-order DMA queues stream W during pooling, not after it) --------
    WP = ctx.enter_context(tc.tile_pool(name="wp", bufs=NCHUNKS))
    wts = []
    for g in range(NCHUNKS):
        cols = CHQ[g] * NCHAN
        wt = WP.tile([128, KT * cols], F32, tag="wt", name=f"wt{g}")
        if g < 6:
            nc.scalar.dma_start(wt[:], w6_d[g])
        else:
            nc.scalar.dma_start(wt[:], w1_d[:])
        wts.append(wt)

    # ---------------- phase 2: AllGather pooled features -------------------
    nc.sync.dma_start(ag_in[:], s_pool[:])
    nc.gpsimd.collective_compute(
        "AllGather", OP.bypass, replica_groups=RG,
        ins=[ag_in.opt()], outs=[ag_out.opt()],
    )
    nc.sync.dma_start(
        pg[:].rearrange("p (c t) -> p c t", c=NCORES),
        ag_out[:].rearrange("c p t -> p c t"),
    )
    # lhsT tiles: pt_all[p, (t, s=(c,b))] = pooled(sample 8c+b, krow t*128+p)
    pt_all = P.tile([128, KT * B], F32, tag="pt_all")
    nc.vector.tensor_copy(
        pt_all[:].rearrange("p (t c b) -> p t c b", t=KT, c=NCORES),
        pg[:].rearrange("p (c b t) -> p t c b", c=NCORES, b=BPC),
    )

    # ---------------- phase 3: sharded matmul + per-chunk postproc ---------
    a2v = a2a_sb[:, : QN * 6].rearrange("b (q c) -> b q c", c=6)
    with tc.tile_pool(name="yps", bufs=6, space="PSUM") as YPS:
        q0 = 0
        for g in range(NCHUNKS):
            nq = CHQ[g]
            cols = nq * NCHAN
            psy = YPS.tile([B, cols], F32, tag="psy", name="psy")
            for k in range(KT):
                nc.tensor.matmul(
                    psy[:], pt_all[:, k * B : (k + 1) * B],
                    wts[g][:, k * cols : (k + 1) * cols],
                    start=(k == 0), stop=(k == KT - 1),
                )
            psv = psy[:].rearrange("b (q c) -> b q c", c=NCHAN)
            # boxes straight into the interleaved AllToAll layout
            nc.vector.tensor_copy(a2v[:, q0 : q0 + nq, 0:4], psv[:, :, 0:4])
            # per-query max score
            nc.vector.tensor_reduce(
                scores[:, q0 : q0 + nq], psv[:, :, 4:NCHAN], axis=AX.X, op=OP.max
            )
            # argmax over classes: first-index ties via iod = 79 - class_id
            eqv = eq[:, : nq * NCL].rearrange("b (q c) -> b q c", c=NCL)
            nc.vector.tensor_tensor(
                eqv, psv[:, :, 4:NCHAN],
                scores[:, q0 : q0 + nq].unsqueeze(-1).broadcast_to((B, nq, NCL)),
                op=OP.is_ge,
            )
            amv = am[:, : nq * NCL].rearrange("b (q c) -> b q c", c=NCL)
            nc.vector.tensor_tensor(
                amv, eqv,
                iod[:B, :].unsqueeze(1).broadcast_to((B, nq, NCL)),
                op=OP.mult,
            )
            nc.vector.tensor_reduce(argt[:, :nq], amv, axis=AX.X, op=OP.max)
            nc.vector.tensor_scalar(
                a2v[:, q0 : q0 + nq, 5], argt[:, :nq], -1.0, float(NCL - 1),
                op0=OP.mult, op1=OP.add,
            )
            q0 += nq

    # ---------------- phase 4: AllToAll per-query results ------------------
    nc.vector.tensor_copy(a2v[:, :, 4], scores[:])
    nc.vector.tensor_copy(a2a_sb[:, QN * 6 :], scores[:])
    nc.sync.dma_start(a2a_in[:], a2a_sb[:])
    nc.gpsimd.collective_compute(
        "AllToAll", OP.bypass, replica_groups=RG,
        ins=[a2a_in.opt()], outs=[a2a_out.opt()],
    )

    # ---------------- phase 5: top-150 tail --------------------------------
    # feat[16b] = sample b's [304, 6] rows (concat of the 8 cores' blocks)
    nc.sync.dma_start(
        feat[:].rearrange("(b s) (c x) -> b s c x", b=BPC, c=NCORES)[:, 0],
        a2a_out[:, :, : QN * 6].rearrange("c b x -> b c x"),
    )
    # swk[b, c*38+q] = score of global query c*38+q for sample b
    nc.scalar.dma_start(
        swk[:].rearrange("b (c q) -> b c q", c=NCORES),
        a2a_out[:, :, QN * 6 :].rearrange("c b q -> b c q"),
    )
    nc.vector.memset(swk[:, NQ:NQP], NEG)  # padded queries never win

    # two-half tail: indices from rounds 0..9 are wrapped + gathered while
    # rounds 10..18 still run on DVE
    def wrap_and_gather(h):
        i0, i1 = h * 80, (h + 1) * 80
        nc.vector.tensor_copy(ti16[:, i0 : min(i1, NROUND * 8)], ti[:, i0 : min(i1, NROUND * 8)])
        nc.scalar.dma_start(tsc[:, i0:i1], ti16[:, i0:i1])
        for b in range(BPC):
            eng = nc.sync if b % 2 == 0 else nc.scalar
            eng.dma_start(
                wraps[h][16 * b : 16 * b + 16, :],
                tsc[b, i0:i1].rearrange("(f p) -> p f", p=16),
            )
        nc.gpsimd.ap_gather(
            gout[:].rearrange("p (i c) -> p i c", c=6)[:, i0:i1],
            feat[:].rearrange("p (q c) -> p q c", c=6),
            wraps[h][:],
            channels=128,
            num_elems=NQP,
            d=6,
            num_idxs=80,
        )

    for r in range(NROUND):
        nc.vector.max(tv[:, 8 * r : 8 * r + 8], swk[:, :])
        nc.vector.max_index(ti[:, 8 * r : 8 * r + 8], tv[:, 8 * r : 8 * r + 8], swk[:, :])
        if r < NROUND - 1:
            nc.vector.match_replace(
                swk[:, :], tv[:, 8 * r : 8 * r + 8], swk[:, :], NEG
            )
        if r == 9:
            wrap_and_gather(0)
    wrap_and_gather(1)

    nc.scalar.dma_start(
        out_d[:].rearrange("b k c -> b (k c)"),
        gout[:].rearrange("(b s) x -> b s x", b=BPC)[:, 0, : TOPK * 6],
    )
    if dbg is not None:
        nc.sync.dma_start(dbg["dti16"][:], ti16[:])
        nc.sync.dma_start(dbg["dwrap"][:, 0:5], wraps[0][:])
        nc.sync.dma_start(dbg["dwrap"][:, 5:10], wraps[1][:])
        nc.sync.dma_start(dbg["dtv"][:], tv[:])
        nc.sync.dma_start(dbg["dgout"][:], gout[:])


def _make_consts():
    iod = np.broadcast_to(
        (np.float32(NCL - 1) - np.arange(NCL, dtype=np.float32))[None, :], (128, NCL)
    ).copy()
    return iod


_NC_CACHE = {}


def _get_nc():
    if "nc" not in _NC_CACHE:
        _NC_CACHE["nc"] = build_program()
    return _NC_CACHE["nc"]


def pack_x(xs: np.ndarray) -> np.ndarray:
    """[BPC, 3, 640, 640] int32 -> [128, BPC*10*1024] uint8 cell-major.

    Cell k = c_rgb*400 + i*20 + j (matching W's row layout after the
    BGR->RGB flip); cell (b, k) sits at partition k%128, free offset
    b*10240 + (k//128)*1024; cells 1200..1279 are zero padding.
    """
    xs8 = xs.astype(np.uint8).reshape(BPC, CHN, 20, 32, 20, 32)
    xs8 = xs8[:, ::-1]  # BGR -> RGB
    cells = xs8.transpose(0, 1, 2, 4, 3, 5).reshape(BPC, KDIM, 1024)
    full = np.zeros((BPC, KPAD, 1024), np.uint8)
    full[:, :KDIM] = cells
    # [b, tl, p, pix] -> [p, b, tl, pix]
    return np.ascontiguousarray(
        full.reshape(BPC, KT, 128, 1024).transpose(2, 0, 1, 3)
    ).reshape(128, BPC * KT * 1024)


def pack_w(W: np.ndarray) -> tuple[np.ndarray, np.ndarray]:
    """[1200, 25200] -> per-core chunk-group tiles (scale folded in).

    Returns (w6 [8, 6, 128, 5040], w1 [8, 128, 1680]): core c, group g holds
    [128 krows, kt, cols] for its query columns, kpad rows 1200..1279 zero.
    """
    Wp = np.zeros((KPAD, NQP * NCHAN), np.float32)
    Wp[:KDIM, : NQ * NCHAN] = (W.astype(np.float64) * SCALE).astype(np.float32)
    w6 = np.zeros((NCORES, 6, 128, KT * CHQ[0] * NCHAN), np.float32)
    w1 = np.zeros((NCORES, 128, KT * CHQ[6] * NCHAN), np.float32)
    for c in range(NCORES):
        s = Wp[:, c * QN * NCHAN : (c + 1) * QN * NCHAN]
        q0 = 0
        for g in range(NCHUNKS):
            cols = CHQ[g] * NCHAN
            blk = s[:, q0 : q0 + cols].reshape(KT, 128, cols).transpose(1, 0, 2)
            if g < 6:
                w6[c, g] = blk.reshape(128, KT * cols)
            else:
                w1[c] = blk.reshape(128, KT * cols)
            q0 += cols
    return w6, w1


def make_in_maps(x: np.ndarray, W: np.ndarray) -> list[dict]:
    iod = _make_consts()
    w6, w1 = pack_w(W)
    in_maps = []
    for c in range(NCORES):
        in_maps.append(
            {
                "x": pack_x(x[c * BPC : (c + 1) * BPC]),
                "w6": w6[c],
                "w1": w1[c],
                "iod": iod,
            }
        )
    return in_maps


def kernel(x: np.ndarray, W: np.ndarray) -> np.ndarray:
    x = np.ascontiguousarray(np.asarray(x), dtype=np.int32)
    W = np.ascontiguousarray(np.asarray(W), dtype=np.float32)
    assert x.shape == (B, CHN, HIMG, WIMG) and W.shape == (KDIM, NOUT)

    nc = _get_nc()
    in_maps = make_in_maps(x, W)
    res = run_bass_kernel_spmd(nc, in_maps, core_ids=list(range(NCORES)))
    out = np.concatenate([res.results[c]["out"] for c in range(NCORES)], axis=0)
    return out.astype(np.float32)


if __name__ == "__main__":
    xs = np.random.randint(0, 256, (B, CHN, HIMG, WIMG)).astype(np.int32)
    Ws = (np.random.randn(KDIM, NOUT) * 0.02).astype(np.float32)
    o = kernel(xs, Ws)
    print("kernel output:", o.shape, o.dtype)
